# revision 3
# baseline (speedup 1.0000x reference)
"""Trainium2 Bass kernel for nn_CXNGeneralLayer (GNN message passing).

z = relu(Gi2j @ (xi W_i + b_i) + Adj2j @ (xj1 W_j1 + b_j1)
         + coAdj2j @ (xj1 W_j2 + b_j2) + Gk2j @ (xk W_k + b_k))

Sharding (per the 1D row-parallel hint): output rows (n_j) are split
across 8 NeuronCores; each core streams its [1024, 8192] shard of all
four operator matrices, which dominate the traffic. The stream is the
bottleneck (HBM ~358 GB/s/core), so the shards are converted to bf16 on
the host (64 MB/core instead of 128 MB; quantization error ~2e-3 abs on
an output scale of ~4.8, far under the 2e-2 gate) and pre-transposed to
[8192(t), 1024(j)] blocks so the contraction dim sits on SBUF
partitions. Blocks are packed so each DMA moves a contiguous 1 MB
[128, 4096] tile (4 t-chunks), alternating between the two HWDGE rings.
The small activations h_m = x_m W_m + b_m are replicated to every core
in bf16 stationary-operand layout, so z^T = sum_m h_m^T @ G_m^T
accumulates directly in PSUM with N=512 moving tiles.
"""

import sys

import numpy as np

if "/opt/trn_rl_repo" not in sys.path:
    sys.path.insert(0, "/opt/trn_rl_repo")

N = 8192  # n_i = n_j = n_k
C = 32  # c_in = c_out
N_CORES = 8
JS = N // N_CORES  # 1024 output rows per core
KP = 128  # contraction partition tile
KCH = N // KP  # 64 t-chunks
TCH = 4  # t-chunks per DMA block (1 MB bf16 per dma_start)
NBLK = KCH // TCH  # 16 blocks per matrix
NJH = 2  # j-halves of 512 (PSUM bank limit for f32 output)

_compiled = None


def _build_program():
    import concourse.mybir as mybir
    import concourse.tile as tile
    from concourse import bacc

    f32 = mybir.dt.float32
    bf16 = mybir.dt.bfloat16
    nc = bacc.Bacc("TRN2", target_bir_lowering=False)

    # G^T shard packed in DMA blocks: gt[b, p, c*JS + j] = G^T[TCH*KP*b + KP*c + p, j]
    gts = [
        nc.dram_tensor(f"gt{m}", [NBLK, KP, TCH * JS], bf16, kind="ExternalInput")
        for m in range(4)
    ]
    # h_m in stationary layout: hs[m][p, 32k+c] = h_m[128k+p, c]
    hs = [
        nc.dram_tensor(f"h{m}", [KP, KCH * C], bf16, kind="ExternalInput")
        for m in range(4)
    ]
    out_t = nc.dram_tensor("outT", [C, JS], f32, kind="ExternalOutput")

    with tile.TileContext(nc) as tc:
        with (
            tc.tile_pool(name="cpool", bufs=1) as cpool,
            tc.tile_pool(name="gpool", bufs=10) as gpool,
            tc.tile_pool(name="zpsum", bufs=2, space="PSUM") as zpsum,
        ):
            h_sb = [
                cpool.tile([KP, KCH * C], bf16, tag=f"h{m}", name=f"h{m}")
                for m in range(4)
            ]

            # z^T[c, j] += sum_t h_m[t, c] * G_m^T[t, j], streaming G^T in
            # 1 MB blocks; one PSUM accumulation group per 512-wide j-half
            # spanning all 4 matrices x 64 chunks. DMAs alternate between
            # the two HWDGE rings; the tiny h tensors ride the same rings
            # ahead of / between the first G blocks so compute starts as
            # early as possible. The final block is split into 256 KB
            # single-chunk pieces to shrink the end-of-stream matmul tail.
            zp = [
                zpsum.tile([C, 512], f32, tag=f"zp{jh}", name=f"zp{jh}")
                for jh in range(NJH)
            ]
            seq = [("h", 0), ("h", 1), ("g", 0), ("g", 1), ("h", 2), ("h", 3)]
            seq += [("g", i) for i in range(2, 4 * NBLK)]
            pos = 0
            for kind, i in seq:
                dma_eng = nc.sync if pos % 2 == 0 else nc.scalar
                pos += 1
                if kind == "h":
                    dma_eng.dma_start(h_sb[i][:], hs[i][:])
                    continue
                m, b = divmod(i, NBLK)
                last_block = i == 4 * NBLK - 1
                # [(ci0, nch), ...] sub-pieces of this block, one DMA each
                pieces = (
                    [(0, TCH)] if not last_block else [(ci, 1) for ci in range(TCH)]
                )
                for pi, (ci0, nch) in enumerate(pieces):
                    if pi > 0:
                        dma_eng = nc.sync if pos % 2 == 0 else nc.scalar
                        pos += 1
                    gt = gpool.tile([KP, nch * JS], bf16, tag=f"gt{nch}")
                    dma_eng.dma_start(
                        gt[:], gts[m][b][:, ci0 * JS : (ci0 + nch) * JS]
                    )
                    for ci in range(nch):
                        k = TCH * b + ci0 + ci
                        first = m == 0 and k == 0
                        last = m == 3 and k == KCH - 1
                        for jh in range(NJH):
                            nc.tensor.matmul(
                                zp[jh][:],
                                h_sb[m][:, C * k : C * (k + 1)],
                                gt[:, JS * ci + 512 * jh : JS * ci + 512 * (jh + 1)],
                                start=first,
                                stop=last,
                            )

            # relu and store z^T shard; per-half so the first store
            # overlaps the other half's final matmul
            zsb = cpool.tile([C, JS], f32, tag="zsb")
            for jh in range(NJH):
                nc.scalar.activation(
                    zsb[:, 512 * jh : 512 * (jh + 1)],
                    zp[jh][:],
                    mybir.ActivationFunctionType.Relu,
                )
                nc.sync.dma_start(
                    out_t[:, 512 * jh : 512 * (jh + 1)],
                    zsb[:, 512 * jh : 512 * (jh + 1)],
                )

    nc.compile()
    return nc


def _get_program():
    global _compiled
    if _compiled is None:
        _compiled = _build_program()
    return _compiled


def _prep_inputs(inputs):
    """Host-side sharding: returns per-core input maps."""
    import ml_dtypes

    bf16 = ml_dtypes.bfloat16
    f32 = np.float32
    branches = [
        ("Gi2j", "xi", "W_i", "b_i"),
        ("Adj2j", "xj1", "W_j1", "b_j1"),
        ("coAdj2j", "xj1", "W_j2", "b_j2"),
        ("Gk2j", "xk", "W_k", "b_k"),
    ]
    shared = {}
    for m, (_, xn, wn, bn) in enumerate(branches):
        x = np.asarray(inputs[xn], dtype=f32)
        w = np.asarray(inputs[wn], dtype=f32)
        b = np.asarray(inputs[bn], dtype=f32)
        h = x @ w + b  # [N, C] replicated activation, broadcast to all cores
        shared[f"h{m}"] = np.ascontiguousarray(
            h.reshape(KCH, KP, C).transpose(1, 0, 2).reshape(KP, KCH * C)
        ).astype(bf16)

    in_maps = []
    for s in range(N_CORES):
        im = dict(shared)
        for m, (gn, _, _, _) in enumerate(branches):
            g = np.asarray(inputs[gn])
            blk = g[s * JS : (s + 1) * JS, :].astype(bf16)  # [JS, N]
            # want gt[b, p, ci*JS + j] = blk[j, TCH*KP*b + KP*ci + p]
            gt = (
                blk.reshape(JS, NBLK, TCH, KP)
                .transpose(1, 3, 2, 0)
                .reshape(NBLK, KP, TCH * JS)
            )
            im[f"gt{m}"] = np.ascontiguousarray(gt)
        in_maps.append(im)
    return in_maps


def _run(inputs, trace=False):
    from concourse.bass_utils import run_bass_kernel_spmd

    nc = _get_program()
    in_maps = _prep_inputs(inputs)
    try:
        res = run_bass_kernel_spmd(nc, in_maps, list(range(N_CORES)), trace=trace)
    except Exception:
        # transient device errors (e.g. NRT_EXEC_UNIT_UNRECOVERABLE) clear
        # on re-dispatch; retry once before giving up
        res = run_bass_kernel_spmd(nc, in_maps, list(range(N_CORES)), trace=trace)
    out = np.concatenate(
        [res.results[s]["outT"] for s in range(N_CORES)], axis=1
    ).T
    return np.ascontiguousarray(out, dtype=np.float32), res


def kernel(**inputs):
    out, _ = _run(inputs, trace=False)
    return out


# revision 5
# speedup vs baseline: 1.1153x; 1.1153x over previous
"""Trainium2 Bass kernel for nn_CXNGeneralLayer (GNN message passing).

z = relu(Gi2j @ (xi W_i + b_i) + Adj2j @ (xj1 W_j1 + b_j1)
         + coAdj2j @ (xj1 W_j2 + b_j2) + Gk2j @ (xk W_k + b_k))

Sharding (per the 1D row-parallel hint): output rows (n_j) are split
across 8 NeuronCores; each core streams its [1024, 8192] shard of all
four operator matrices, which dominate the traffic. The stream is the
bottleneck (HBM ~358 GB/s/core), so the shards are converted to bf16 on
the host (64 MB/core instead of 128 MB; quantization error ~2e-3 abs on
an output scale of ~4.8, far under the 2e-2 gate) and pre-transposed to
[8192(t), 1024(j)] blocks so the contraction dim sits on SBUF
partitions. Blocks are packed so each DMA moves a contiguous 1 MB
[128, 4096] tile (4 t-chunks), alternating between the two HWDGE rings.
The small activations h_m = x_m W_m + b_m are replicated to every core
in bf16 stationary-operand layout, so z^T = sum_m h_m^T @ G_m^T
accumulates directly in PSUM with N=512 moving tiles.
"""

import sys

import numpy as np

if "/opt/trn_rl_repo" not in sys.path:
    sys.path.insert(0, "/opt/trn_rl_repo")

N = 8192  # n_i = n_j = n_k
C = 32  # c_in = c_out
N_CORES = 8
JS = N // N_CORES  # 1024 output rows per core
KP = 128  # contraction partition tile
KCH = N // KP  # 64 t-chunks
TCH = 2  # t-chunks per DMA block (512 KB bf16 per dma_start)
NBLK = KCH // TCH  # 16 blocks per matrix
NJH = 2  # j-halves of 512 (PSUM bank limit for f32 output)

_compiled = None


def _build_program():
    import concourse.mybir as mybir
    import concourse.tile as tile
    from concourse import bacc

    f32 = mybir.dt.float32
    bf16 = mybir.dt.bfloat16
    nc = bacc.Bacc("TRN2", target_bir_lowering=False)

    # G^T shard packed in DMA blocks: gt[b, p, c*JS + j] = G^T[TCH*KP*b + KP*c + p, j]
    gts = [
        nc.dram_tensor(f"gt{m}", [NBLK, KP, TCH * JS], bf16, kind="ExternalInput")
        for m in range(4)
    ]
    # h_m in stationary layout: hs[m][p, 32k+c] = h_m[128k+p, c]
    hs = [
        nc.dram_tensor(f"h{m}", [KP, KCH * C], bf16, kind="ExternalInput")
        for m in range(4)
    ]
    out_t = nc.dram_tensor("outT", [C, JS], f32, kind="ExternalOutput")

    with tile.TileContext(nc) as tc:
        with (
            tc.tile_pool(name="cpool", bufs=1) as cpool,
            tc.tile_pool(name="gpool", bufs=20) as gpool,
            tc.tile_pool(name="zpsum", bufs=2, space="PSUM") as zpsum,
        ):
            h_sb = []
            for m in range(4):
                h = cpool.tile([KP, KCH * C], bf16, tag=f"h{m}", name=f"h{m}")
                # SWDGE queue: keeps the HWDGE rings free for the G stream;
                # the slow h0 arrival also builds a multi-block matmul
                # backlog, which keeps the PE continuously busy (HAM warm)
                # for the rest of the stream.
                nc.gpsimd.dma_start(h[:], hs[m][:])
                h_sb.append(h)

            # z^T[c, j] += sum_t h_m[t, c] * G_m^T[t, j], streaming G^T in
            # 512 KB blocks; one PSUM accumulation group per 512-wide j-half
            # spanning all 4 matrices x 64 chunks. DMAs alternate between
            # the two HWDGE rings; the deep pool (10 MB) keeps the stream
            # at full rate while the PE is still cold/waiting. The final
            # block is split into 256 KB single-chunk pieces to shrink the
            # end-of-stream matmul tail.
            zp = [
                zpsum.tile([C, 512], f32, tag=f"zp{jh}", name=f"zp{jh}")
                for jh in range(NJH)
            ]
            pos = 0
            for i in range(4 * NBLK):
                dma_eng = nc.sync if pos % 2 == 0 else nc.scalar
                pos += 1
                m, b = divmod(i, NBLK)
                last_block = i == 4 * NBLK - 1
                # [(ci0, nch), ...] sub-pieces of this block, one DMA each
                pieces = (
                    [(0, TCH)] if not last_block else [(ci, 1) for ci in range(TCH)]
                )
                for pi, (ci0, nch) in enumerate(pieces):
                    if pi > 0:
                        dma_eng = nc.sync if pos % 2 == 0 else nc.scalar
                        pos += 1
                    gt = gpool.tile([KP, nch * JS], bf16, tag=f"gt{nch}")
                    dma_eng.dma_start(
                        gt[:], gts[m][b][:, ci0 * JS : (ci0 + nch) * JS]
                    )
                    for ci in range(nch):
                        k = TCH * b + ci0 + ci
                        first = m == 0 and k == 0
                        last = m == 3 and k == KCH - 1
                        for jh in range(NJH):
                            nc.tensor.matmul(
                                zp[jh][:],
                                h_sb[m][:, C * k : C * (k + 1)],
                                gt[:, JS * ci + 512 * jh : JS * ci + 512 * (jh + 1)],
                                start=first,
                                stop=last,
                            )

            # relu and store z^T shard; per-half so the first store
            # overlaps the other half's final matmul
            zsb = cpool.tile([C, JS], f32, tag="zsb")
            for jh in range(NJH):
                nc.scalar.activation(
                    zsb[:, 512 * jh : 512 * (jh + 1)],
                    zp[jh][:],
                    mybir.ActivationFunctionType.Relu,
                )
                nc.sync.dma_start(
                    out_t[:, 512 * jh : 512 * (jh + 1)],
                    zsb[:, 512 * jh : 512 * (jh + 1)],
                )

    nc.compile()
    return nc


def _get_program():
    global _compiled
    if _compiled is None:
        _compiled = _build_program()
    return _compiled


def _prep_inputs(inputs):
    """Host-side sharding: returns per-core input maps."""
    import ml_dtypes

    bf16 = ml_dtypes.bfloat16
    f32 = np.float32
    branches = [
        ("Gi2j", "xi", "W_i", "b_i"),
        ("Adj2j", "xj1", "W_j1", "b_j1"),
        ("coAdj2j", "xj1", "W_j2", "b_j2"),
        ("Gk2j", "xk", "W_k", "b_k"),
    ]
    shared = {}
    for m, (_, xn, wn, bn) in enumerate(branches):
        x = np.asarray(inputs[xn], dtype=f32)
        w = np.asarray(inputs[wn], dtype=f32)
        b = np.asarray(inputs[bn], dtype=f32)
        h = x @ w + b  # [N, C] replicated activation, broadcast to all cores
        shared[f"h{m}"] = np.ascontiguousarray(
            h.reshape(KCH, KP, C).transpose(1, 0, 2).reshape(KP, KCH * C)
        ).astype(bf16)

    in_maps = []
    for s in range(N_CORES):
        im = dict(shared)
        for m, (gn, _, _, _) in enumerate(branches):
            g = np.asarray(inputs[gn])
            blk = g[s * JS : (s + 1) * JS, :].astype(bf16)  # [JS, N]
            # want gt[b, p, ci*JS + j] = blk[j, TCH*KP*b + KP*ci + p]
            gt = (
                blk.reshape(JS, NBLK, TCH, KP)
                .transpose(1, 3, 2, 0)
                .reshape(NBLK, KP, TCH * JS)
            )
            im[f"gt{m}"] = np.ascontiguousarray(gt)
        in_maps.append(im)
    return in_maps


def _run(inputs, trace=False):
    from concourse.bass_utils import run_bass_kernel_spmd

    nc = _get_program()
    in_maps = _prep_inputs(inputs)
    try:
        res = run_bass_kernel_spmd(nc, in_maps, list(range(N_CORES)), trace=trace)
    except Exception:
        # transient device errors (e.g. NRT_EXEC_UNIT_UNRECOVERABLE) clear
        # on re-dispatch; retry once before giving up
        res = run_bass_kernel_spmd(nc, in_maps, list(range(N_CORES)), trace=trace)
    out = np.concatenate(
        [res.results[s]["outT"] for s in range(N_CORES)], axis=1
    ).T
    return np.ascontiguousarray(out, dtype=np.float32), res


def kernel(**inputs):
    out, _ = _run(inputs, trace=False)
    return out
